# revision 3
# baseline (speedup 1.0000x reference)
"""Paged-attention decode (vLLM single_query_cached_kv_attention +
reshape_and_cache) for Trainium2, 8 NeuronCores.

Strategy
--------
Sequences are sharded across the 8 cores (4 per core), sorted by context
length so each "slot" (per-core sequence index) has a similar length on
every core; one SPMD program is built with a per-slot group count
G = ceil((L-1)/128) taken as the max over the 8 cores of that slot.

Per (slot, head) on each core:
  * K cache blocks are DMA'd contiguously (512B descriptors) into an SBUF
    tile laid out [128=(block_lane nb, d_outer do), G*128=(group g,(pos,di))].
  * Scores are computed by 8 accumulating TensorE matmuls (one per d_inner
    di) whose stationary operand is a block-diagonal matrix holding
    SCALE*q (8 diagonal copies of q[do,di], one per block lane) - this
    contracts (nb,do) partitions correctly with no transposes of K.
  * Softmax: DVE adds a host-built mask (-1e9 for pos >= L and the stale
    pos == L-1 slot), ACT computes exp with fused per-lane row sums.
  * attn^T via one small PE transpose; V accumulated with 8 matmuls
    against the naturally-laid-out V tile [(g,pos), d].
  * The new token's k/v (reshape_and_cache) is folded in exactly via a
    batched side path: e_new = exp(SCALE*q.k_new) joins the softmax sum
    and e_new*v_new joins the output accumulation.
Outputs are normalized by the reciprocal of the exp-sum and gathered.
"""
import sys

for _p in ("/opt/trn_rl_repo", "/root/.axon_site/_ro/trn_rl_repo"):
    if _p not in sys.path:
        sys.path.insert(0, _p)

import numpy as np
import concourse.bass as bass
import concourse.mybir as mybir
import concourse.tile as tile
from concourse.bass_utils import run_bass_kernel_spmd

F32 = mybir.dt.float32
AF = mybir.ActivationFunctionType
ALU = mybir.AluOpType

SCALE = 0.08838834764831845  # 1/sqrt(128)
B, H, D, BS, NB, X, MAX_BLOCKS = 32, 16, 128, 16, 2048, 8, 64
N_CORES = 8
SLOTS = B // N_CORES  # 4


def split_multi_waits(nc):
    """This walrus build rejects instructions with more than one sync wait;
    move extra waits onto preceding same-engine NoOps (equivalent: an
    engine's queue executes sequentially, so a wait on the NoOp still
    gates the following instruction)."""
    for f in nc.m.functions:
        for blk in f.blocks:
            new = []
            for ins in blk.instructions:
                si = ins.sync_info
                if si is not None and len(si.on_wait) > 1:
                    waits = list(si.on_wait)
                    for w in waits[:-1]:
                        nop = mybir.InstNoOp(
                            name=f"waitsplit-{nc.next_id()}",
                            engine=ins.engine, ins=[], outs=[])
                        nop.sync_info = mybir.SyncInfo(on_wait=[w], on_update=[])
                        new.append(nop)
                    si.on_wait = waits[-1:]
                new.append(ins)
            blk.instructions = new


def build_program(G_slots, n_heads=H, dev_sim=False):
    """Single SPMD program. G_slots[s] = #groups of (8 blocks x 16 pos)."""
    n_slots = len(G_slots)
    nblk_tot = 8 * sum(G_slots)
    offs = np.cumsum([0] + [8 * g for g in G_slots])
    NSH = n_slots * n_heads

    nc = bass.Bass()
    kc = nc.declare_dram_parameter("kc", [nblk_tot, n_heads, 2048], F32, isOutput=False)
    vc = nc.declare_dram_parameter("vc", [nblk_tot, n_heads, 16, 128], F32, isOutput=False)
    bdq = nc.declare_dram_parameter("bdq", [128, NSH * 64], F32, isOutput=False)
    msk = nc.declare_dram_parameter("msk", [8, NSH * 128], F32, isOutput=False)
    qh = nc.declare_dram_parameter("qh", [NSH, 128], F32, isOutput=False)
    kn = nc.declare_dram_parameter("kn", [NSH, 128], F32, isOutput=False)
    vn = nc.declare_dram_parameter("vn", [1, NSH * 128], F32, isOutput=False)
    ident = nc.declare_dram_parameter("ident", [64, 64], F32, isOutput=False)
    ones = nc.declare_dram_parameter("ones", [64, 1], F32, isOutput=False)
    out = nc.declare_dram_parameter("out", [1, NSH * 128], F32, isOutput=True)

    with tile.TileContext(nc) as tc:
        with (
            tc.tile_pool(name="const", bufs=1) as cpool,
            tc.tile_pool(name="kx", bufs=3) as kpool,
            tc.tile_pool(name="vx", bufs=3) as vpool,
            tc.tile_pool(name="sm", bufs=3) as spool,
            tc.tile_pool(name="ps_s", bufs=2, space="PSUM") as ps_s_pool,
            tc.tile_pool(name="ps_t", bufs=2, space="PSUM") as ps_t_pool,
            tc.tile_pool(name="ps_o", bufs=2, space="PSUM") as ps_o_pool,
        ):
            # ---- constants + batched new-token side path ----
            t_bdq = cpool.tile([128, NSH * 64], F32, tag="bdq")
            nc.sync.dma_start(t_bdq[:], bdq[:])
            t_msk = cpool.tile([8, NSH * 128], F32, tag="msk")
            nc.sync.dma_start(t_msk[:], msk[:])
            t_id = cpool.tile([64, 64], F32, tag="ident")
            nc.sync.dma_start(t_id[:], ident[:])
            t_ones = cpool.tile([64, 1], F32, tag="ones")
            nc.sync.dma_start(t_ones[:], ones[:])
            t_vn = cpool.tile([1, NSH * 128], F32, tag="vn")
            nc.sync.dma_start(t_vn[:], vn[:])
            t_qh = cpool.tile([NSH, 128], F32, tag="qh")
            nc.sync.dma_start(t_qh[:], qh[:])
            t_kn = cpool.tile([NSH, 128], F32, tag="kn")
            nc.sync.dma_start(t_kn[:], kn[:])

            t_prod = cpool.tile([NSH, 128], F32, tag="prod")
            t_snew = cpool.tile([NSH, 1], F32, tag="snew")
            nc.vector.tensor_mul(t_prod[:], t_qh[:], t_kn[:])
            nc.vector.reduce_sum(t_snew[:], t_prod[:], axis=mybir.AxisListType.X)
            t_enew = cpool.tile([NSH, 1], F32, tag="enew")
            nc.scalar.activation(t_enew[:], t_snew[:], AF.Exp, scale=SCALE)
            ps_en = ps_s_pool.tile([8, 128], F32, tag="scores")
            nc.tensor.transpose(ps_en[0:1, 0:NSH], t_enew[:], t_id[0:NSH, 0:NSH])
            t_enew_r = cpool.tile([1, NSH], F32, tag="enewr")
            nc.vector.tensor_copy(t_enew_r[:], ps_en[0:1, 0:NSH])

            t_out = cpool.tile([1, NSH * 128], F32, tag="outrow")

            # ---- per (slot, head) ----
            for s in range(n_slots):
                G = G_slots[s]
                blk0 = int(offs[s])
                for h in range(n_heads):
                    sh = s * n_heads + h
                    # K tile [128=(nb,do), G*128]; one DMA per block lane nb
                    # (plain unit-step partition APs; 512B descriptors)
                    t_k = kpool.tile([128, G * 128], F32, tag="ktile")
                    if dev_sim:
                        nc.gpsimd.memset(t_k[:], 0.0)
                    for nb in range(8):
                        kin = kc[blk0 + nb: blk0 + 8 * G: 8, h, :].rearrange(
                            "g (do pd) -> do g pd", pd=128)
                        kout = t_k[nb * 16:(nb + 1) * 16, :].rearrange(
                            "do (g pd) -> do g pd", pd=128)
                        nc.sync.dma_start(kout, kin)
                    # V tile [(g,pos)=G*16, (nb,d)=1024]
                    t_v = vpool.tile([G * 16, 8 * 128], F32, tag="vtile")
                    if dev_sim:
                        nc.gpsimd.memset(t_v[:], 0.0)
                    for g in range(G):
                        vin = vc[blk0 + 8 * g: blk0 + 8 * g + 8, h, :, :].rearrange(
                            "nb p d -> p nb d")
                        vout_ = t_v[g * 16:(g + 1) * 16, :].rearrange(
                            "p (nb d) -> p nb d", d=128)
                        nc.sync.dma_start(vout_, vin)

                    # scores [8, G*16] = 8 accumulating di-matmuls
                    ps_sc = ps_s_pool.tile([8, 128], F32, tag="scores")
                    k_r = t_k[:].rearrange("k (g p di) -> k g p di", p=16, di=8)
                    for di in range(8):
                        nc.tensor.matmul(
                            ps_sc[:, 0:G * 16],
                            t_bdq[:, sh * 64 + di * 8: sh * 64 + di * 8 + 8],
                            k_r[:, :, :, di],
                            start=(di == 0), stop=(di == 7))

                    # mask-add then exp with fused row sums
                    t_ms = spool.tile([8, 128], F32, tag="msc")
                    nc.vector.tensor_add(
                        t_ms[:, 0:G * 16], ps_sc[:, 0:G * 16],
                        t_msk[:, sh * 128: sh * 128 + G * 16])
                    t_ex = spool.tile([8, 128], F32, tag="exps")
                    t_sum = spool.tile([8, 1], F32, tag="sums")
                    nc.scalar.activation(
                        t_ex[:, 0:G * 16], t_ms[:, 0:G * 16], AF.Exp,
                        accum_out=t_sum[:])

                    # attn^T [G*16, 8]
                    ps_at = ps_t_pool.tile([128, 8], F32, tag="attnT")
                    nc.tensor.transpose(ps_at[0:G * 16, :], t_ex[:, 0:G * 16],
                                        t_id[0:8, 0:8])
                    t_at = spool.tile([128, 8], F32, tag="attnTs")
                    nc.vector.tensor_copy(t_at[0:G * 16, :], ps_at[0:G * 16, :])

                    # out_unnorm [1,128] and total [1,1] share one PSUM tile
                    ps_o = ps_o_pool.tile([1, 256], F32, tag="vout")
                    v_r = t_v[:].rearrange("gp (nb d) -> gp nb d", nb=8)
                    for nb in range(8):
                        nc.tensor.matmul(
                            ps_o[:, 0:128],
                            t_at[0:G * 16, nb:nb + 1],
                            v_r[:, nb, :],
                            start=(nb == 0), stop=False, skip_group_check=True)
                    nc.tensor.matmul(
                        ps_o[:, 0:128],
                        t_enew_r[:, sh:sh + 1],
                        t_vn[:, sh * 128:(sh + 1) * 128],
                        start=False, stop=True, skip_group_check=True)
                    nc.tensor.matmul(
                        ps_o[:, 128:129], t_ones[0:8, :], t_sum[:],
                        start=True, stop=False, skip_group_check=True)
                    nc.tensor.matmul(
                        ps_o[:, 128:129], t_enew_r[:, sh:sh + 1], t_ones[0:1, :],
                        start=False, stop=True, skip_group_check=True)

                    t_rec = spool.tile([1, 1], F32, tag="rec")
                    nc.vector.reciprocal(t_rec[:], ps_o[:, 128:129])
                    nc.vector.tensor_scalar_mul(
                        t_out[:, sh * 128:(sh + 1) * 128], ps_o[:, 0:128], t_rec[:])

            nc.sync.dma_start(out[:], t_out[:])

    return nc


def _host_inputs(G_slots, seq_ids_by_core, query, key, value, key_cache,
                 value_cache, block_tables, context_lens):
    """Per-core input maps. seq_ids_by_core[c][s] = sequence index."""
    n_slots = len(G_slots)
    NSH = n_slots * H
    key_cache = np.asarray(key_cache)
    value_cache = np.asarray(value_cache)
    block_tables = np.asarray(block_tables)
    context_lens = np.asarray(context_lens)
    query = np.asarray(query)
    key = np.asarray(key)
    value = np.asarray(value)

    # shared constants
    ident = np.eye(64, dtype=np.float32)
    ones_arr = np.ones((64, 1), np.float32)

    # per-slot position-validity pattern container
    g_idx = np.arange(8)
    pos_idx = np.arange(16)
    nb_idx = np.arange(8)
    p_grid = (g_idx[None, :, None] * 8 + nb_idx[:, None, None]) * 16 \
        + pos_idx[None, None, :]  # (nb, g, pos)

    in_maps = []
    for c in range(N_CORES):
        ids = seq_ids_by_core[c]
        # gather cache blocks, slot-concatenated
        blk_rows = np.concatenate(
            [block_tables[ids[s], 0:8 * G_slots[s]] for s in range(n_slots)])
        kc = key_cache[blk_rows]          # [N, H, 16, 16, 8]
        vc = value_cache[blk_rows]        # [N, H, 16, 128]
        kc = np.ascontiguousarray(kc.reshape(kc.shape[0], H, 2048))
        vc = np.ascontiguousarray(vc)

        q_rows = query[ids]               # [n_slots, H, 128]
        kn_rows = key[ids]
        vn_rows = value[ids]
        lens = context_lens[ids]

        qv = q_rows.reshape(n_slots, H, 16, 8)
        bdq = np.zeros((8, 16, n_slots, H, 8, 8), np.float32)
        for nb in range(8):
            bdq[nb, :, :, :, :, nb] = SCALE * qv.transpose(2, 0, 1, 3)
        bdq = np.ascontiguousarray(bdq.reshape(128, NSH * 64))

        msk = np.empty((8, n_slots, H, 8, 16), np.float32)
        for s in range(n_slots):
            L = int(lens[s])
            valid = (p_grid < L) & (p_grid != L - 1)
            msk[:, s, :, :, :] = np.where(valid, 0.0, -1e9)[:, None, :, :]
        msk = np.ascontiguousarray(msk.reshape(8, NSH * 128))

        in_maps.append(dict(
            kc=kc, vc=vc, bdq=bdq, msk=msk,
            qh=np.ascontiguousarray(q_rows.reshape(NSH, 128)),
            kn=np.ascontiguousarray(kn_rows.reshape(NSH, 128)),
            vn=np.ascontiguousarray(vn_rows.reshape(1, NSH * 128)),
            ident=ident, ones=ones_arr,
        ))
    return in_maps


def _plan(context_lens):
    """Assign sequences to (core, slot) sorted by length; per-slot G."""
    lens = np.asarray(context_lens)
    order = np.argsort(-lens, kind="stable")  # longest first
    seq_ids_by_core = [[0] * SLOTS for _ in range(N_CORES)]
    G_slots = []
    for s in range(SLOTS):
        chunk = order[s * N_CORES:(s + 1) * N_CORES]
        for c in range(N_CORES):
            seq_ids_by_core[c][s] = int(chunk[c])
        Lmax = int(lens[chunk].max())
        G_slots.append(max(1, -(-(Lmax - 1) // 128)))  # ceil((L-1)/128)
    return tuple(G_slots), seq_ids_by_core


def kernel(query, key, value, key_cache, value_cache, block_tables,
           context_lens, slot_mapping, _run=None):
    G_slots, seq_ids_by_core = _plan(context_lens)
    nc = build_program(G_slots)
    split_multi_waits(nc)
    in_maps = _host_inputs(G_slots, seq_ids_by_core, query, key, value,
                           key_cache, value_cache, block_tables, context_lens)
    runner = _run or (lambda nc_, maps: run_bass_kernel_spmd(
        nc_, maps, core_ids=list(range(N_CORES))).results)
    results = runner(nc, in_maps)

    out = np.empty((B, H * D), np.float32)
    for c in range(N_CORES):
        row = results[c]["out"].reshape(SLOTS * H * D)
        for s in range(SLOTS):
            i = seq_ids_by_core[c][s]
            out[i] = row[s * H * D:(s + 1) * H * D]
    return out


# revision 5
# speedup vs baseline: 1.1966x; 1.1966x over previous
"""Paged-attention decode (vLLM single_query_cached_kv_attention +
reshape_and_cache) for Trainium2, 8 NeuronCores.

Strategy
--------
Sequences are sharded across the 8 cores (4 per core), sorted by context
length so each "slot" (per-core sequence index) has a similar length on
every core; one SPMD program is built with a per-slot group count
G = ceil((L-1)/128) taken as the max over the 8 cores of that slot.

Per (slot, head) on each core:
  * K cache blocks are DMA'd contiguously (512B descriptors) into an SBUF
    tile laid out [128=(block_lane nb, d_outer do), G*128=(group g,(pos,di))].
  * Scores are computed by 8 accumulating TensorE matmuls (one per d_inner
    di) whose stationary operand is a block-diagonal matrix holding
    SCALE*q (8 diagonal copies of q[do,di], one per block lane) - this
    contracts (nb,do) partitions correctly with no transposes of K.
  * Softmax: DVE adds a host-built mask (-1e9 for pos >= L and the stale
    pos == L-1 slot), ACT computes exp with fused per-lane row sums.
  * attn^T via one small PE transpose; V accumulated with 8 matmuls
    against the naturally-laid-out V tile [(g,pos), d].
  * The new token's k/v (reshape_and_cache) is folded in exactly via a
    batched side path: e_new = exp(SCALE*q.k_new) joins the softmax sum
    and e_new*v_new joins the output accumulation.
Outputs are normalized by the reciprocal of the exp-sum and gathered.
"""
import sys

for _p in ("/opt/trn_rl_repo", "/root/.axon_site/_ro/trn_rl_repo"):
    if _p not in sys.path:
        sys.path.insert(0, _p)

import numpy as np
from ml_dtypes import bfloat16 as ml_bfloat16
import concourse.bass as bass
import concourse.mybir as mybir
import concourse.tile as tile
from concourse.bass_utils import run_bass_kernel_spmd

F32 = mybir.dt.float32
BF16 = mybir.dt.bfloat16
AF = mybir.ActivationFunctionType
ALU = mybir.AluOpType

SCALE = 0.08838834764831845  # 1/sqrt(128)
B, H, D, BS, NB, X, MAX_BLOCKS = 32, 16, 128, 16, 2048, 8, 64
N_CORES = 8
SLOTS = B // N_CORES  # 4


def split_multi_waits(nc):
    """This walrus build rejects instructions with more than one sync wait;
    move extra waits onto preceding same-engine NoOps (equivalent: an
    engine's queue executes sequentially, so a wait on the NoOp still
    gates the following instruction)."""
    for f in nc.m.functions:
        for blk in f.blocks:
            new = []
            for ins in blk.instructions:
                si = ins.sync_info
                if si is not None and len(si.on_wait) > 1:
                    waits = list(si.on_wait)
                    for w in waits[:-1]:
                        nop = mybir.InstNoOp(
                            name=f"waitsplit-{nc.next_id()}",
                            engine=ins.engine, ins=[], outs=[])
                        nop.sync_info = mybir.SyncInfo(on_wait=[w], on_update=[])
                        new.append(nop)
                    si.on_wait = waits[-1:]
                new.append(ins)
            blk.instructions = new


def build_program(G_slots, n_heads=H, dev_sim=False):
    """Single SPMD program. G_slots[s] = #groups of (8 blocks x 16 pos)."""
    n_slots = len(G_slots)
    nblk_tot = 8 * sum(G_slots)
    offs = np.cumsum([0] + [8 * g for g in G_slots])
    NSH = n_slots * n_heads

    nc = bass.Bass()
    kc = nc.declare_dram_parameter("kc", [nblk_tot, n_heads, 2048], F32, isOutput=False)
    vc = nc.declare_dram_parameter("vc", [nblk_tot, n_heads, 16, 128], F32, isOutput=False)
    bdq = nc.declare_dram_parameter("bdq", [128, NSH * 64], BF16, isOutput=False)
    msk = nc.declare_dram_parameter("msk", [8, NSH * 128], F32, isOutput=False)
    qh = nc.declare_dram_parameter("qh", [NSH, 128], F32, isOutput=False)
    kn = nc.declare_dram_parameter("kn", [NSH, 128], F32, isOutput=False)
    vn = nc.declare_dram_parameter("vn", [1, NSH * 128], BF16, isOutput=False)
    ident = nc.declare_dram_parameter("ident", [64, 64], F32, isOutput=False)
    identb = nc.declare_dram_parameter("identb", [8, 8], BF16, isOutput=False)
    ones = nc.declare_dram_parameter("ones", [64, 1], F32, isOutput=False)
    out = nc.declare_dram_parameter("out", [1, NSH * 128], F32, isOutput=True)

    with tile.TileContext(nc) as tc:
        with (
            tc.tile_pool(name="const", bufs=1) as cpool,
            tc.tile_pool(name="kx", bufs=3) as kpool,
            tc.tile_pool(name="kb", bufs=3) as kbpool,
            tc.tile_pool(name="vx", bufs=3) as vpool,
            tc.tile_pool(name="vb", bufs=3) as vbpool,
            tc.tile_pool(name="sm", bufs=3) as spool,
            tc.tile_pool(name="ps_s", bufs=2, space="PSUM") as ps_s_pool,
            tc.tile_pool(name="ps_t", bufs=2, space="PSUM") as ps_t_pool,
            tc.tile_pool(name="ps_o", bufs=2, space="PSUM") as ps_o_pool,
        ):
            # ---- constants + batched new-token side path ----
            t_bdq = cpool.tile([128, NSH * 64], BF16, tag="bdq")
            nc.sync.dma_start(t_bdq[:], bdq[:])
            t_msk = cpool.tile([8, NSH * 128], F32, tag="msk")
            nc.sync.dma_start(t_msk[:], msk[:])
            t_id = cpool.tile([64, 64], F32, tag="ident")
            nc.sync.dma_start(t_id[:], ident[:])
            t_idb = cpool.tile([8, 8], BF16, tag="identb")
            nc.sync.dma_start(t_idb[:], identb[:])
            t_ones = cpool.tile([64, 1], F32, tag="ones")
            nc.sync.dma_start(t_ones[:], ones[:])
            t_vn = cpool.tile([1, NSH * 128], BF16, tag="vn")
            nc.sync.dma_start(t_vn[:], vn[:])
            t_qh = cpool.tile([NSH, 128], F32, tag="qh")
            nc.sync.dma_start(t_qh[:], qh[:])
            t_kn = cpool.tile([NSH, 128], F32, tag="kn")
            nc.sync.dma_start(t_kn[:], kn[:])

            t_prod = cpool.tile([NSH, 128], F32, tag="prod")
            t_snew = cpool.tile([NSH, 1], F32, tag="snew")
            nc.vector.tensor_mul(t_prod[:], t_qh[:], t_kn[:])
            nc.vector.reduce_sum(t_snew[:], t_prod[:], axis=mybir.AxisListType.X)
            t_enew = cpool.tile([NSH, 1], F32, tag="enew")
            nc.scalar.activation(t_enew[:], t_snew[:], AF.Exp, scale=SCALE)
            ps_en = ps_s_pool.tile([8, 128], F32, tag="scores")
            nc.tensor.transpose(ps_en[0:1, 0:NSH], t_enew[:], t_id[0:NSH, 0:NSH])
            t_enew_r = cpool.tile([1, NSH], F32, tag="enewr")
            nc.vector.tensor_copy(t_enew_r[:], ps_en[0:1, 0:NSH])
            t_enew_rb = cpool.tile([1, NSH], BF16, tag="enewrb")
            nc.vector.tensor_copy(t_enew_rb[:], ps_en[0:1, 0:NSH])

            t_out = cpool.tile([1, NSH * 128], F32, tag="outrow")

            # ---- per (slot, head) ----
            for s in range(n_slots):
                G = G_slots[s]
                blk0 = int(offs[s])
                for h in range(n_heads):
                    sh = s * n_heads + h
                    # K tile [128=(nb,do), G*128]; one DMA per block lane nb
                    # (plain unit-step partition APs; 512B descriptors)
                    t_k = kpool.tile([128, G * 128], F32, tag="ktile")
                    if dev_sim:
                        nc.gpsimd.memset(t_k[:], 0.0)
                    for nb in range(8):
                        kin = kc[blk0 + nb: blk0 + 8 * G: 8, h, :].rearrange(
                            "g (do pd) -> do g pd", pd=128)
                        kout = t_k[nb * 16:(nb + 1) * 16, :].rearrange(
                            "do (g pd) -> do g pd", pd=128)
                        nc.sync.dma_start(kout, kin)
                    t_kb = kbpool.tile([128, G * 128], BF16, tag="kbtile")
                    nc.vector.tensor_copy(t_kb[:], t_k[:])
                    # V tile [(g,pos)=G*16, (nb,d)=1024]
                    t_v = vpool.tile([G * 16, 8 * 128], F32, tag="vtile")
                    if dev_sim:
                        nc.gpsimd.memset(t_v[:], 0.0)
                    for g in range(G):
                        vin = vc[blk0 + 8 * g: blk0 + 8 * g + 8, h, :, :].rearrange(
                            "nb p d -> p nb d")
                        vout_ = t_v[g * 16:(g + 1) * 16, :].rearrange(
                            "p (nb d) -> p nb d", d=128)
                        nc.scalar.dma_start(vout_, vin)
                    t_vb = vbpool.tile([G * 16, 8 * 128], BF16, tag="vbtile")
                    nc.gpsimd.tensor_copy(t_vb[:], t_v[:])

                    # scores [8, G*16] = 8 accumulating di-matmuls
                    ps_sc = ps_s_pool.tile([8, 128], F32, tag="scores")
                    k_r = t_kb[:].rearrange("k (g p di) -> k g p di", p=16, di=8)
                    for di in range(8):
                        nc.tensor.matmul(
                            ps_sc[:, 0:G * 16],
                            t_bdq[:, sh * 64 + di * 8: sh * 64 + di * 8 + 8],
                            k_r[:, :, :, di],
                            start=(di == 0), stop=(di == 7))

                    # mask-add then exp with fused row sums
                    t_ms = spool.tile([8, 128], F32, tag="msc")
                    nc.vector.tensor_add(
                        t_ms[:, 0:G * 16], ps_sc[:, 0:G * 16],
                        t_msk[:, sh * 128: sh * 128 + G * 16])
                    t_ex = spool.tile([8, 128], BF16, tag="exps")
                    t_sum = spool.tile([8, 1], F32, tag="sums")
                    nc.scalar.activation(
                        t_ex[:, 0:G * 16], t_ms[:, 0:G * 16], AF.Exp,
                        accum_out=t_sum[:])

                    # attn^T [G*16, 8]
                    ps_at = ps_t_pool.tile([128, 8], BF16, tag="attnT")
                    nc.tensor.transpose(ps_at[0:G * 16, :], t_ex[:, 0:G * 16],
                                        t_idb[:, :])
                    t_at = spool.tile([128, 8], BF16, tag="attnTs")
                    nc.vector.tensor_copy(t_at[0:G * 16, :], ps_at[0:G * 16, :])

                    # out_unnorm [1,128] and total [1,1] share one PSUM tile
                    ps_o = ps_o_pool.tile([1, 256], F32, tag="vout")
                    v_r = t_vb[:].rearrange("gp (nb d) -> gp nb d", nb=8)
                    for nb in range(8):
                        nc.tensor.matmul(
                            ps_o[:, 0:128],
                            t_at[0:G * 16, nb:nb + 1],
                            v_r[:, nb, :],
                            start=(nb == 0), stop=False, skip_group_check=True)
                    nc.tensor.matmul(
                        ps_o[:, 0:128],
                        t_enew_rb[:, sh:sh + 1],
                        t_vn[:, sh * 128:(sh + 1) * 128],
                        start=False, stop=True, skip_group_check=True)
                    nc.tensor.matmul(
                        ps_o[:, 128:129], t_ones[0:8, :], t_sum[:],
                        start=True, stop=False, skip_group_check=True)
                    nc.tensor.matmul(
                        ps_o[:, 128:129], t_enew_r[:, sh:sh + 1], t_ones[0:1, :],
                        start=False, stop=True, skip_group_check=True)

                    t_rec = spool.tile([1, 1], F32, tag="rec")
                    nc.vector.reciprocal(t_rec[:], ps_o[:, 128:129])
                    nc.vector.tensor_scalar_mul(
                        t_out[:, sh * 128:(sh + 1) * 128], ps_o[:, 0:128], t_rec[:])

            nc.sync.dma_start(out[:], t_out[:])

    return nc


def _host_inputs(G_slots, seq_ids_by_core, query, key, value, key_cache,
                 value_cache, block_tables, context_lens):
    """Per-core input maps. seq_ids_by_core[c][s] = sequence index."""
    n_slots = len(G_slots)
    NSH = n_slots * H
    key_cache = np.asarray(key_cache)
    value_cache = np.asarray(value_cache)
    block_tables = np.asarray(block_tables)
    context_lens = np.asarray(context_lens)
    query = np.asarray(query)
    key = np.asarray(key)
    value = np.asarray(value)

    # shared constants
    ident = np.eye(64, dtype=np.float32)
    identb = np.eye(8, dtype=np.float32).astype(ml_bfloat16)
    ones_arr = np.ones((64, 1), np.float32)

    # per-slot position-validity pattern container
    g_idx = np.arange(8)
    pos_idx = np.arange(16)
    nb_idx = np.arange(8)
    p_grid = (g_idx[None, :, None] * 8 + nb_idx[:, None, None]) * 16 \
        + pos_idx[None, None, :]  # (nb, g, pos)

    in_maps = []
    for c in range(N_CORES):
        ids = seq_ids_by_core[c]
        # gather cache blocks, slot-concatenated
        blk_rows = np.concatenate(
            [block_tables[ids[s], 0:8 * G_slots[s]] for s in range(n_slots)])
        kc = key_cache[blk_rows]          # [N, H, 16, 16, 8]
        vc = value_cache[blk_rows]        # [N, H, 16, 128]
        kc = np.ascontiguousarray(kc.reshape(kc.shape[0], H, 2048))
        vc = np.ascontiguousarray(vc)

        q_rows = query[ids]               # [n_slots, H, 128]
        kn_rows = key[ids]
        vn_rows = value[ids]
        lens = context_lens[ids]

        qv = q_rows.reshape(n_slots, H, 16, 8)
        bdq = np.zeros((8, 16, n_slots, H, 8, 8), np.float32)
        for nb in range(8):
            bdq[nb, :, :, :, :, nb] = SCALE * qv.transpose(2, 0, 1, 3)
        bdq = np.ascontiguousarray(
            bdq.reshape(128, NSH * 64)).astype(ml_bfloat16)

        msk = np.empty((8, n_slots, H, 8, 16), np.float32)
        for s in range(n_slots):
            L = int(lens[s])
            valid = (p_grid < L) & (p_grid != L - 1)
            msk[:, s, :, :, :] = np.where(valid, 0.0, -1e9)[:, None, :, :]
        msk = np.ascontiguousarray(msk.reshape(8, NSH * 128))

        in_maps.append(dict(
            kc=kc, vc=vc, bdq=bdq, msk=msk,
            qh=np.ascontiguousarray(q_rows.reshape(NSH, 128)),
            kn=np.ascontiguousarray(kn_rows.reshape(NSH, 128)),
            vn=np.ascontiguousarray(
                vn_rows.reshape(1, NSH * 128)).astype(ml_bfloat16),
            ident=ident, identb=identb, ones=ones_arr,
        ))
    return in_maps


def _plan(context_lens):
    """Assign sequences to (core, slot) sorted by length; per-slot G."""
    lens = np.asarray(context_lens)
    order = np.argsort(-lens, kind="stable")  # longest first
    seq_ids_by_core = [[0] * SLOTS for _ in range(N_CORES)]
    G_slots = []
    for s in range(SLOTS):
        chunk = order[s * N_CORES:(s + 1) * N_CORES]
        for c in range(N_CORES):
            seq_ids_by_core[c][s] = int(chunk[c])
        Lmax = int(lens[chunk].max())
        G_slots.append(max(1, -(-(Lmax - 1) // 128)))  # ceil((L-1)/128)
    return tuple(G_slots), seq_ids_by_core


def kernel(query, key, value, key_cache, value_cache, block_tables,
           context_lens, slot_mapping, _run=None):
    G_slots, seq_ids_by_core = _plan(context_lens)
    nc = build_program(G_slots)
    split_multi_waits(nc)
    in_maps = _host_inputs(G_slots, seq_ids_by_core, query, key, value,
                           key_cache, value_cache, block_tables, context_lens)
    runner = _run or (lambda nc_, maps: run_bass_kernel_spmd(
        nc_, maps, core_ids=list(range(N_CORES))).results)
    results = runner(nc, in_maps)

    out = np.empty((B, H * D), np.float32)
    for c in range(N_CORES):
        row = results[c]["out"].reshape(SLOTS * H * D)
        for s in range(SLOTS):
            i = seq_ids_by_core[c][s]
            out[i] = row[s * H * D:(s + 1) * H * D]
    return out


# revision 7
# speedup vs baseline: 1.3495x; 1.1278x over previous
"""Paged-attention decode (vLLM single_query_cached_kv_attention +
reshape_and_cache) for Trainium2, 8 NeuronCores.

Strategy
--------
Sequences are sharded across the 8 cores (4 per core), sorted by context
length so each "slot" (per-core sequence index) has a similar length on
every core; one SPMD program is built with a per-slot group count
G = ceil((L-1)/128) taken as the max over the 8 cores of that slot.

Per (slot, head) on each core:
  * K cache blocks are DMA'd contiguously (512B descriptors) into an SBUF
    tile laid out [128=(block_lane nb, d_outer do), G*128=(group g,(pos,di))].
  * Scores are computed by 8 accumulating TensorE matmuls (one per d_inner
    di) whose stationary operand is a block-diagonal matrix holding
    SCALE*q (8 diagonal copies of q[do,di], one per block lane) - this
    contracts (nb,do) partitions correctly with no transposes of K.
  * Softmax: DVE adds a host-built mask (-1e9 for pos >= L and the stale
    pos == L-1 slot), ACT computes exp with fused per-lane row sums.
  * attn^T via one small PE transpose; V accumulated with 8 matmuls
    against the naturally-laid-out V tile [(g,pos), d].
  * The new token's k/v (reshape_and_cache) is folded in exactly via a
    batched side path: e_new = exp(SCALE*q.k_new) joins the softmax sum
    and e_new*v_new joins the output accumulation.
Outputs are normalized by the reciprocal of the exp-sum and gathered.
"""
import sys

for _p in ("/opt/trn_rl_repo", "/root/.axon_site/_ro/trn_rl_repo"):
    if _p not in sys.path:
        sys.path.insert(0, _p)

import numpy as np
import concourse.bass as bass
import concourse.mybir as mybir
import concourse.tile as tile
from concourse.bass_utils import run_bass_kernel_spmd

F32 = mybir.dt.float32
F32R = mybir.dt.float32r
AF = mybir.ActivationFunctionType
ALU = mybir.AluOpType

SCALE = 0.08838834764831845  # 1/sqrt(128)
B, H, D, BS, NB, X, MAX_BLOCKS = 32, 16, 128, 16, 2048, 8, 64
N_CORES = 8
SLOTS = B // N_CORES  # 4


def split_multi_waits(nc):
    """This walrus build rejects instructions with more than one sync wait;
    move extra waits onto preceding same-engine NoOps (equivalent: an
    engine's queue executes sequentially, so a wait on the NoOp still
    gates the following instruction)."""
    for f in nc.m.functions:
        for blk in f.blocks:
            new = []
            for ins in blk.instructions:
                si = ins.sync_info
                if si is not None and len(si.on_wait) > 1:
                    waits = list(si.on_wait)
                    for w in waits[:-1]:
                        nop = mybir.InstNoOp(
                            name=f"waitsplit-{nc.next_id()}",
                            engine=ins.engine, ins=[], outs=[])
                        nop.sync_info = mybir.SyncInfo(on_wait=[w], on_update=[])
                        new.append(nop)
                    si.on_wait = waits[-1:]
                new.append(ins)
            blk.instructions = new


def build_program(G_slots, n_heads=H, dev_sim=False):
    """Single SPMD program. G_slots[s] = #groups of (8 blocks x 16 pos)."""
    n_slots = len(G_slots)
    nblk_tot = 8 * sum(G_slots)
    offs = np.cumsum([0] + [8 * g for g in G_slots])
    NSH = n_slots * n_heads

    nc = bass.Bass()
    kc = nc.declare_dram_parameter("kc", [nblk_tot, n_heads, 2048], F32, isOutput=False)
    vc = nc.declare_dram_parameter("vc", [nblk_tot, n_heads, 16, 128], F32, isOutput=False)
    bdq = nc.declare_dram_parameter("bdq", [128, NSH * 64], F32R, isOutput=False)
    msk = nc.declare_dram_parameter("msk", [8, NSH * 128], F32, isOutput=False)
    qh = nc.declare_dram_parameter("qh", [NSH, 128], F32, isOutput=False)
    kn = nc.declare_dram_parameter("kn", [NSH, 128], F32, isOutput=False)
    vn = nc.declare_dram_parameter("vn", [1, NSH * 128], F32R, isOutput=False)
    ident = nc.declare_dram_parameter("ident", [64, 64], F32, isOutput=False)
    ones = nc.declare_dram_parameter("ones", [64, 1], F32, isOutput=False)
    out = nc.declare_dram_parameter("out", [1, NSH * 128], F32, isOutput=True)

    with tile.TileContext(nc) as tc:
        with (
            tc.tile_pool(name="const", bufs=1) as cpool,
            tc.tile_pool(name="kx", bufs=4) as kpool,
            tc.tile_pool(name="vx", bufs=4) as vpool,
            tc.tile_pool(name="sm", bufs=4) as spool,
            tc.tile_pool(name="ps_s", bufs=3, space="PSUM") as ps_s_pool,
            tc.tile_pool(name="ps_t", bufs=2, space="PSUM") as ps_t_pool,
            tc.tile_pool(name="ps_o", bufs=3, space="PSUM") as ps_o_pool,
        ):
            # ---- constants + batched new-token side path ----
            t_bdq = cpool.tile([128, NSH * 64], F32R, tag="bdq")
            nc.sync.dma_start(t_bdq[:], bdq[:])
            t_msk = cpool.tile([8, NSH * 128], F32, tag="msk")
            nc.sync.dma_start(t_msk[:], msk[:])
            t_id = cpool.tile([64, 64], F32, tag="ident")
            nc.sync.dma_start(t_id[:], ident[:])
            t_ones = cpool.tile([64, 1], F32, tag="ones")
            nc.sync.dma_start(t_ones[:], ones[:])
            t_vn = cpool.tile([1, NSH * 128], F32R, tag="vn")
            nc.sync.dma_start(t_vn[:], vn[:])
            t_qh = cpool.tile([NSH, 128], F32, tag="qh")
            nc.sync.dma_start(t_qh[:], qh[:])
            t_kn = cpool.tile([NSH, 128], F32, tag="kn")
            nc.sync.dma_start(t_kn[:], kn[:])

            t_prod = cpool.tile([NSH, 128], F32, tag="prod")
            t_snew = cpool.tile([NSH, 1], F32, tag="snew")
            nc.vector.tensor_mul(t_prod[:], t_qh[:], t_kn[:])
            nc.vector.reduce_sum(t_snew[:], t_prod[:], axis=mybir.AxisListType.X)
            t_enew = cpool.tile([NSH, 1], F32, tag="enew")
            nc.scalar.activation(t_enew[:], t_snew[:], AF.Exp, scale=SCALE)
            ps_en = ps_s_pool.tile([8, 128], F32, tag="scores")
            nc.tensor.transpose(ps_en[0:1, 0:NSH], t_enew[:], t_id[0:NSH, 0:NSH])
            t_enew_r = cpool.tile([1, NSH], F32, tag="enewr")
            nc.vector.tensor_copy(t_enew_r[:], ps_en[0:1, 0:NSH])
            t_enew_rr = cpool.tile([1, NSH], F32R, tag="enewrr")
            nc.vector.tensor_copy(t_enew_rr[:], ps_en[0:1, 0:NSH])

            t_out = cpool.tile([1, NSH * 128], F32, tag="outrow")

            # ---- per (slot, head) ----
            for s in range(n_slots):
                G = G_slots[s]
                blk0 = int(offs[s])
                for h in range(n_heads):
                    sh = s * n_heads + h
                    # K tile [128=(nb,do), G*128]; one DMA per block lane nb
                    # (plain unit-step partition APs; 512B descriptors)
                    t_k = kpool.tile([128, G * 128], F32R, tag="ktile")
                    if dev_sim:
                        nc.gpsimd.memset(t_k[:], 0.0)
                    for nb in range(8):
                        kin = kc[blk0 + nb: blk0 + 8 * G: 8, h, :].bitcast(
                            F32R).rearrange("g (do pd) -> do g pd", pd=128)
                        kout = t_k[nb * 16:(nb + 1) * 16, :].rearrange(
                            "do (g pd) -> do g pd", pd=128)
                        (nc.sync if nb < 4 else nc.scalar).dma_start(kout, kin)
                    # V tile [(g,pos)=G*16, (nb,d)=1024]
                    t_v = vpool.tile([G * 16, 8 * 128], F32R, tag="vtile")
                    if dev_sim:
                        nc.gpsimd.memset(t_v[:], 0.0)
                    for g in range(G):
                        vin = vc[blk0 + 8 * g: blk0 + 8 * g + 8, h, :, :].bitcast(
                            F32R).rearrange("nb p d -> p nb d")
                        vout_ = t_v[g * 16:(g + 1) * 16, :].rearrange(
                            "p (nb d) -> p nb d", d=128)
                        (nc.sync if g % 2 else nc.scalar).dma_start(vout_, vin)

                    # scores [8, G*16] = 8 accumulating di-matmuls
                    ps_sc = ps_s_pool.tile([8, 128], F32, tag="scores")
                    k_r = t_k[:].rearrange("k (g p di) -> k g p di", p=16, di=8)
                    for di in range(8):
                        nc.tensor.matmul(
                            ps_sc[:, 0:G * 16],
                            t_bdq[:, sh * 64 + di * 8: sh * 64 + di * 8 + 8],
                            k_r[:, :, :, di],
                            start=(di == 0), stop=(di == 7))

                    # mask-add then exp with fused row sums
                    t_ms = spool.tile([8, 128], F32, tag="msc")
                    nc.vector.tensor_add(
                        t_ms[:, 0:G * 16], ps_sc[:, 0:G * 16],
                        t_msk[:, sh * 128: sh * 128 + G * 16])
                    t_ex = spool.tile([8, 128], F32, tag="exps")
                    t_sum = spool.tile([8, 1], F32, tag="sums")
                    nc.scalar.activation(
                        t_ex[:, 0:G * 16], t_ms[:, 0:G * 16], AF.Exp,
                        accum_out=t_sum[:])

                    # attn^T [G*16, 8]
                    ps_at = ps_t_pool.tile([128, 8], F32, tag="attnT")
                    nc.tensor.transpose(ps_at[0:G * 16, :], t_ex[:, 0:G * 16],
                                        t_id[0:8, 0:8])
                    t_at = spool.tile([128, 8], F32R, tag="attnTs")
                    nc.vector.tensor_copy(t_at[0:G * 16, :], ps_at[0:G * 16, :])

                    # out_unnorm [1,128] and total [1,1] share one PSUM tile
                    ps_o = ps_o_pool.tile([1, 256], F32, tag="vout")
                    v_r = t_v[:].rearrange("gp (nb d) -> gp nb d", nb=8)
                    for nb in range(8):
                        nc.tensor.matmul(
                            ps_o[:, 0:128],
                            t_at[0:G * 16, nb:nb + 1],
                            v_r[:, nb, :],
                            start=(nb == 0), stop=False, skip_group_check=True)
                    nc.tensor.matmul(
                        ps_o[:, 0:128],
                        t_enew_rr[:, sh:sh + 1],
                        t_vn[:, sh * 128:(sh + 1) * 128],
                        start=False, stop=True, skip_group_check=True)
                    nc.tensor.matmul(
                        ps_o[:, 128:129], t_ones[0:8, :], t_sum[:],
                        start=True, stop=False, skip_group_check=True)
                    nc.tensor.matmul(
                        ps_o[:, 128:129], t_enew_r[:, sh:sh + 1], t_ones[0:1, :],
                        start=False, stop=True, skip_group_check=True)

                    t_rec = spool.tile([1, 1], F32, tag="rec")
                    nc.vector.reciprocal(t_rec[:], ps_o[:, 128:129])
                    nc.vector.tensor_scalar_mul(
                        t_out[:, sh * 128:(sh + 1) * 128], ps_o[:, 0:128], t_rec[:])

            nc.sync.dma_start(out[:], t_out[:])

    return nc


def _host_inputs(G_slots, seq_ids_by_core, query, key, value, key_cache,
                 value_cache, block_tables, context_lens):
    """Per-core input maps. seq_ids_by_core[c][s] = sequence index."""
    n_slots = len(G_slots)
    NSH = n_slots * H
    key_cache = np.asarray(key_cache)
    value_cache = np.asarray(value_cache)
    block_tables = np.asarray(block_tables)
    context_lens = np.asarray(context_lens)
    query = np.asarray(query)
    key = np.asarray(key)
    value = np.asarray(value)

    # shared constants
    ident = np.eye(64, dtype=np.float32)
    ones_arr = np.ones((64, 1), np.float32)

    # per-slot position-validity pattern container
    g_idx = np.arange(8)
    pos_idx = np.arange(16)
    nb_idx = np.arange(8)
    p_grid = (g_idx[None, :, None] * 8 + nb_idx[:, None, None]) * 16 \
        + pos_idx[None, None, :]  # (nb, g, pos)

    in_maps = []
    for c in range(N_CORES):
        ids = seq_ids_by_core[c]
        # gather cache blocks, slot-concatenated
        blk_rows = np.concatenate(
            [block_tables[ids[s], 0:8 * G_slots[s]] for s in range(n_slots)])
        kc = key_cache[blk_rows]          # [N, H, 16, 16, 8]
        vc = value_cache[blk_rows]        # [N, H, 16, 128]
        kc = np.ascontiguousarray(kc.reshape(kc.shape[0], H, 2048))
        vc = np.ascontiguousarray(vc)

        q_rows = query[ids]               # [n_slots, H, 128]
        kn_rows = key[ids]
        vn_rows = value[ids]
        lens = context_lens[ids]

        qv = q_rows.reshape(n_slots, H, 16, 8)
        bdq = np.zeros((8, 16, n_slots, H, 8, 8), np.float32)
        for nb in range(8):
            bdq[nb, :, :, :, :, nb] = SCALE * qv.transpose(2, 0, 1, 3)
        bdq = np.ascontiguousarray(bdq.reshape(128, NSH * 64))

        msk = np.empty((8, n_slots, H, 8, 16), np.float32)
        for s in range(n_slots):
            L = int(lens[s])
            valid = (p_grid < L) & (p_grid != L - 1)
            msk[:, s, :, :, :] = np.where(valid, 0.0, -1e9)[:, None, :, :]
        msk = np.ascontiguousarray(msk.reshape(8, NSH * 128))

        in_maps.append(dict(
            kc=kc, vc=vc, bdq=bdq, msk=msk,
            qh=np.ascontiguousarray(q_rows.reshape(NSH, 128)),
            kn=np.ascontiguousarray(kn_rows.reshape(NSH, 128)),
            vn=np.ascontiguousarray(vn_rows.reshape(1, NSH * 128)),
            ident=ident, ones=ones_arr,
        ))
    return in_maps


def _plan(context_lens):
    """Assign sequences to (core, slot) sorted by length; per-slot G."""
    lens = np.asarray(context_lens)
    order = np.argsort(-lens, kind="stable")  # longest first
    seq_ids_by_core = [[0] * SLOTS for _ in range(N_CORES)]
    G_slots = []
    for s in range(SLOTS):
        chunk = order[s * N_CORES:(s + 1) * N_CORES]
        for c in range(N_CORES):
            seq_ids_by_core[c][s] = int(chunk[c])
        Lmax = int(lens[chunk].max())
        G_slots.append(max(1, -(-(Lmax - 1) // 128)))  # ceil((L-1)/128)
    return tuple(G_slots), seq_ids_by_core


def kernel(query, key, value, key_cache, value_cache, block_tables,
           context_lens, slot_mapping, _run=None):
    G_slots, seq_ids_by_core = _plan(context_lens)
    nc = build_program(G_slots)
    split_multi_waits(nc)
    in_maps = _host_inputs(G_slots, seq_ids_by_core, query, key, value,
                           key_cache, value_cache, block_tables, context_lens)
    runner = _run or (lambda nc_, maps: run_bass_kernel_spmd(
        nc_, maps, core_ids=list(range(N_CORES))).results)
    results = runner(nc, in_maps)

    out = np.empty((B, H * D), np.float32)
    for c in range(N_CORES):
        row = results[c]["out"].reshape(SLOTS * H * D)
        for s in range(SLOTS):
            i = seq_ids_by_core[c][s]
            out[i] = row[s * H * D:(s + 1) * H * D]
    return out


# revision 8
# speedup vs baseline: 1.5523x; 1.1503x over previous
"""Paged-attention decode (vLLM single_query_cached_kv_attention +
reshape_and_cache) for Trainium2, 8 NeuronCores.

Strategy
--------
Sequences are sharded across the 8 cores (4 per core), sorted by context
length so each "slot" (per-core sequence index) has a similar length on
every core; one SPMD program is built with a per-slot group count
G = ceil((L-1)/128) taken as the max over the 8 cores of that slot.

Per (slot, head) on each core:
  * K cache blocks are DMA'd contiguously (512B descriptors) into an SBUF
    tile laid out [128=(block_lane nb, d_outer do), G*128=(group g,(pos,di))].
  * Scores are computed by 8 accumulating TensorE matmuls (one per d_inner
    di) whose stationary operand is a block-diagonal matrix holding
    SCALE*q (8 diagonal copies of q[do,di], one per block lane) - this
    contracts (nb,do) partitions correctly with no transposes of K.
  * Softmax: DVE adds a host-built mask (-1e9 for pos >= L and the stale
    pos == L-1 slot), ACT computes exp with fused per-lane row sums.
  * attn^T via one small PE transpose; V accumulated with 8 matmuls
    against the naturally-laid-out V tile [(g,pos), d].
  * The new token's k/v (reshape_and_cache) is folded in exactly via a
    batched side path: e_new = exp(SCALE*q.k_new) joins the softmax sum
    and e_new*v_new joins the output accumulation.
Outputs are normalized by the reciprocal of the exp-sum and gathered.
"""
import sys

for _p in ("/opt/trn_rl_repo", "/root/.axon_site/_ro/trn_rl_repo"):
    if _p not in sys.path:
        sys.path.insert(0, _p)

import numpy as np
import concourse.bass as bass
import concourse.mybir as mybir
import concourse.tile as tile
from concourse.bass_utils import run_bass_kernel_spmd

F32 = mybir.dt.float32
F32R = mybir.dt.float32r
AF = mybir.ActivationFunctionType
ALU = mybir.AluOpType

SCALE = 0.08838834764831845  # 1/sqrt(128)
B, H, D, BS, NB, X, MAX_BLOCKS = 32, 16, 128, 16, 2048, 8, 64
N_CORES = 8
SLOTS = B // N_CORES  # 4


def split_multi_waits(nc):
    """This walrus build rejects instructions with more than one sync wait;
    move extra waits onto preceding same-engine NoOps (equivalent: an
    engine's queue executes sequentially, so a wait on the NoOp still
    gates the following instruction)."""
    for f in nc.m.functions:
        for blk in f.blocks:
            new = []
            for ins in blk.instructions:
                si = ins.sync_info
                if si is not None and len(si.on_wait) > 1:
                    waits = list(si.on_wait)
                    for w in waits[:-1]:
                        nop = mybir.InstNoOp(
                            name=f"waitsplit-{nc.next_id()}",
                            engine=ins.engine, ins=[], outs=[])
                        nop.sync_info = mybir.SyncInfo(on_wait=[w], on_update=[])
                        new.append(nop)
                    si.on_wait = waits[-1:]
                new.append(ins)
            blk.instructions = new


def build_program(G_slots, n_heads=H, dev_sim=False):
    """Single SPMD program. G_slots[s] = #groups of (8 blocks x 16 pos)."""
    n_slots = len(G_slots)
    nblk_tot = 8 * sum(G_slots)
    offs = np.cumsum([0] + [8 * g for g in G_slots])
    NSH = n_slots * n_heads

    nc = bass.Bass()
    kc = nc.declare_dram_parameter("kc", [nblk_tot, n_heads, 2048], F32, isOutput=False)
    vc = nc.declare_dram_parameter("vc", [nblk_tot, n_heads, 16, 128], F32, isOutput=False)
    bdq = nc.declare_dram_parameter("bdq", [128, NSH * 64], F32R, isOutput=False)
    msk = nc.declare_dram_parameter("msk", [8, NSH * 128], F32, isOutput=False)
    qh = nc.declare_dram_parameter("qh", [NSH, 128], F32, isOutput=False)
    kn = nc.declare_dram_parameter("kn", [NSH, 128], F32, isOutput=False)
    vn = nc.declare_dram_parameter("vn", [1, NSH * 128], F32R, isOutput=False)
    ident = nc.declare_dram_parameter("ident", [64, 64], F32, isOutput=False)
    ones = nc.declare_dram_parameter("ones", [64, 1], F32, isOutput=False)
    out = nc.declare_dram_parameter("out", [1, NSH * 128], F32, isOutput=True)

    with tile.TileContext(nc) as tc:
        with (
            tc.tile_pool(name="const", bufs=1) as cpool,
            tc.tile_pool(name="kx", bufs=4) as kpool,
            tc.tile_pool(name="vx", bufs=4) as vpool,
            tc.tile_pool(name="sm", bufs=4) as spool,
            tc.tile_pool(name="ps_s", bufs=3, space="PSUM") as ps_s_pool,
            tc.tile_pool(name="ps_t", bufs=2, space="PSUM") as ps_t_pool,
            tc.tile_pool(name="ps_o", bufs=3, space="PSUM") as ps_o_pool,
        ):
            # ---- constants + batched new-token side path ----
            t_bdq = cpool.tile([128, NSH * 64], F32R, tag="bdq")
            nc.sync.dma_start(t_bdq[:], bdq[:])
            t_msk = cpool.tile([8, NSH * 128], F32, tag="msk")
            nc.sync.dma_start(t_msk[:], msk[:])
            t_id = cpool.tile([64, 64], F32, tag="ident")
            nc.sync.dma_start(t_id[:], ident[:])
            t_ones = cpool.tile([64, 1], F32, tag="ones")
            nc.sync.dma_start(t_ones[:], ones[:])
            t_vn = cpool.tile([1, NSH * 128], F32R, tag="vn")
            nc.sync.dma_start(t_vn[:], vn[:])
            t_qh = cpool.tile([NSH, 128], F32, tag="qh")
            nc.sync.dma_start(t_qh[:], qh[:])
            t_kn = cpool.tile([NSH, 128], F32, tag="kn")
            nc.sync.dma_start(t_kn[:], kn[:])

            t_prod = cpool.tile([NSH, 128], F32, tag="prod")
            t_snew = cpool.tile([NSH, 1], F32, tag="snew")
            nc.vector.tensor_mul(t_prod[:], t_qh[:], t_kn[:])
            nc.vector.reduce_sum(t_snew[:], t_prod[:], axis=mybir.AxisListType.X)
            t_enew = cpool.tile([NSH, 1], F32, tag="enew")
            nc.scalar.activation(t_enew[:], t_snew[:], AF.Exp, scale=SCALE)
            ps_en = ps_s_pool.tile([8, 128], F32, tag="scores")
            nc.tensor.transpose(ps_en[0:1, 0:NSH], t_enew[:], t_id[0:NSH, 0:NSH])
            t_enew_r = cpool.tile([1, NSH], F32, tag="enewr")
            nc.vector.tensor_copy(t_enew_r[:], ps_en[0:1, 0:NSH])
            t_enew_rr = cpool.tile([1, NSH], F32R, tag="enewrr")
            nc.vector.tensor_copy(t_enew_rr[:], ps_en[0:1, 0:NSH])

            t_out = cpool.tile([1, NSH * 128], F32, tag="outrow")

            # ---- per (slot, head) ----
            for s in range(n_slots):
                G = G_slots[s]
                blk0 = int(offs[s])
                for h in range(n_heads):
                    sh = s * n_heads + h
                    # K tile [128=(nb,do), G*128]; one DMA per block lane nb
                    # (plain unit-step partition APs; 512B descriptors)
                    t_k = kpool.tile([128, G * 128], F32R, tag="ktile")
                    if dev_sim:
                        nc.gpsimd.memset(t_k[:], 0.0)
                    for nb in range(8):
                        kin = kc[blk0 + nb: blk0 + 8 * G: 8, h, :].bitcast(
                            F32R).rearrange("g (do pd) -> do g pd", pd=128)
                        kout = t_k[nb * 16:(nb + 1) * 16, :].rearrange(
                            "do (g pd) -> do g pd", pd=128)
                        _keng = (nc.sync, nc.scalar, nc.gpsimd)[
                            0 if nb < 3 else (1 if nb < 6 else 2)]
                        _keng.dma_start(kout, kin)
                    # V tile [(g,pos)=G*16, (nb,d)=1024]
                    t_v = vpool.tile([G * 16, 8 * 128], F32R, tag="vtile")
                    if dev_sim:
                        nc.gpsimd.memset(t_v[:], 0.0)
                    for g in range(G):
                        vin = vc[blk0 + 8 * g: blk0 + 8 * g + 8, h, :, :].bitcast(
                            F32R).rearrange("nb p d -> p nb d")
                        vout_ = t_v[g * 16:(g + 1) * 16, :].rearrange(
                            "p (nb d) -> p nb d", d=128)
                        (nc.sync, nc.scalar, nc.gpsimd)[g % 3].dma_start(vout_, vin)

                    # scores [8, G*16] = 8 accumulating di-matmuls
                    ps_sc = ps_s_pool.tile([8, 128], F32, tag="scores")
                    k_r = t_k[:].rearrange("k (g p di) -> k g p di", p=16, di=8)
                    for di in range(8):
                        nc.tensor.matmul(
                            ps_sc[:, 0:G * 16],
                            t_bdq[:, sh * 64 + di * 8: sh * 64 + di * 8 + 8],
                            k_r[:, :, :, di],
                            start=(di == 0), stop=(di == 7))

                    # mask-add then exp with fused row sums
                    t_ms = spool.tile([8, 128], F32, tag="msc")
                    nc.vector.tensor_add(
                        t_ms[:, 0:G * 16], ps_sc[:, 0:G * 16],
                        t_msk[:, sh * 128: sh * 128 + G * 16])
                    t_ex = spool.tile([8, 128], F32, tag="exps")
                    t_sum = spool.tile([8, 1], F32, tag="sums")
                    nc.scalar.activation(
                        t_ex[:, 0:G * 16], t_ms[:, 0:G * 16], AF.Exp,
                        accum_out=t_sum[:])

                    # attn^T [G*16, 8]
                    ps_at = ps_t_pool.tile([128, 8], F32, tag="attnT")
                    nc.tensor.transpose(ps_at[0:G * 16, :], t_ex[:, 0:G * 16],
                                        t_id[0:8, 0:8])
                    t_at = spool.tile([128, 8], F32R, tag="attnTs")
                    nc.vector.tensor_copy(t_at[0:G * 16, :], ps_at[0:G * 16, :])

                    # out_unnorm [1,128] and total [1,1] share one PSUM tile
                    ps_o = ps_o_pool.tile([1, 256], F32, tag="vout")
                    v_r = t_v[:].rearrange("gp (nb d) -> gp nb d", nb=8)
                    for nb in range(8):
                        nc.tensor.matmul(
                            ps_o[:, 0:128],
                            t_at[0:G * 16, nb:nb + 1],
                            v_r[:, nb, :],
                            start=(nb == 0), stop=False, skip_group_check=True)
                    nc.tensor.matmul(
                        ps_o[:, 0:128],
                        t_enew_rr[:, sh:sh + 1],
                        t_vn[:, sh * 128:(sh + 1) * 128],
                        start=False, stop=True, skip_group_check=True)
                    nc.tensor.matmul(
                        ps_o[:, 128:129], t_ones[0:8, :], t_sum[:],
                        start=True, stop=False, skip_group_check=True)
                    nc.tensor.matmul(
                        ps_o[:, 128:129], t_enew_r[:, sh:sh + 1], t_ones[0:1, :],
                        start=False, stop=True, skip_group_check=True)

                    t_rec = spool.tile([1, 1], F32, tag="rec")
                    nc.vector.reciprocal(t_rec[:], ps_o[:, 128:129])
                    nc.vector.tensor_scalar_mul(
                        t_out[:, sh * 128:(sh + 1) * 128], ps_o[:, 0:128], t_rec[:])

            nc.sync.dma_start(out[:], t_out[:])

    return nc


def _host_inputs(G_slots, seq_ids_by_core, query, key, value, key_cache,
                 value_cache, block_tables, context_lens):
    """Per-core input maps. seq_ids_by_core[c][s] = sequence index."""
    n_slots = len(G_slots)
    NSH = n_slots * H
    key_cache = np.asarray(key_cache)
    value_cache = np.asarray(value_cache)
    block_tables = np.asarray(block_tables)
    context_lens = np.asarray(context_lens)
    query = np.asarray(query)
    key = np.asarray(key)
    value = np.asarray(value)

    # shared constants
    ident = np.eye(64, dtype=np.float32)
    ones_arr = np.ones((64, 1), np.float32)

    # per-slot position-validity pattern container
    g_idx = np.arange(8)
    pos_idx = np.arange(16)
    nb_idx = np.arange(8)
    p_grid = (g_idx[None, :, None] * 8 + nb_idx[:, None, None]) * 16 \
        + pos_idx[None, None, :]  # (nb, g, pos)

    in_maps = []
    for c in range(N_CORES):
        ids = seq_ids_by_core[c]
        # gather cache blocks, slot-concatenated
        blk_rows = np.concatenate(
            [block_tables[ids[s], 0:8 * G_slots[s]] for s in range(n_slots)])
        kc = key_cache[blk_rows]          # [N, H, 16, 16, 8]
        vc = value_cache[blk_rows]        # [N, H, 16, 128]
        kc = np.ascontiguousarray(kc.reshape(kc.shape[0], H, 2048))
        vc = np.ascontiguousarray(vc)

        q_rows = query[ids]               # [n_slots, H, 128]
        kn_rows = key[ids]
        vn_rows = value[ids]
        lens = context_lens[ids]

        qv = q_rows.reshape(n_slots, H, 16, 8)
        bdq = np.zeros((8, 16, n_slots, H, 8, 8), np.float32)
        for nb in range(8):
            bdq[nb, :, :, :, :, nb] = SCALE * qv.transpose(2, 0, 1, 3)
        bdq = np.ascontiguousarray(bdq.reshape(128, NSH * 64))

        msk = np.empty((8, n_slots, H, 8, 16), np.float32)
        for s in range(n_slots):
            L = int(lens[s])
            valid = (p_grid < L) & (p_grid != L - 1)
            msk[:, s, :, :, :] = np.where(valid, 0.0, -1e9)[:, None, :, :]
        msk = np.ascontiguousarray(msk.reshape(8, NSH * 128))

        in_maps.append(dict(
            kc=kc, vc=vc, bdq=bdq, msk=msk,
            qh=np.ascontiguousarray(q_rows.reshape(NSH, 128)),
            kn=np.ascontiguousarray(kn_rows.reshape(NSH, 128)),
            vn=np.ascontiguousarray(vn_rows.reshape(1, NSH * 128)),
            ident=ident, ones=ones_arr,
        ))
    return in_maps


def _plan(context_lens):
    """Assign sequences to (core, slot) sorted by length; per-slot G."""
    lens = np.asarray(context_lens)
    order = np.argsort(-lens, kind="stable")  # longest first
    seq_ids_by_core = [[0] * SLOTS for _ in range(N_CORES)]
    G_slots = []
    for s in range(SLOTS):
        chunk = order[s * N_CORES:(s + 1) * N_CORES]
        for c in range(N_CORES):
            seq_ids_by_core[c][s] = int(chunk[c])
        Lmax = int(lens[chunk].max())
        G_slots.append(max(1, -(-(Lmax - 1) // 128)))  # ceil((L-1)/128)
    return tuple(G_slots), seq_ids_by_core


def kernel(query, key, value, key_cache, value_cache, block_tables,
           context_lens, slot_mapping, _run=None):
    G_slots, seq_ids_by_core = _plan(context_lens)
    nc = build_program(G_slots)
    split_multi_waits(nc)
    in_maps = _host_inputs(G_slots, seq_ids_by_core, query, key, value,
                           key_cache, value_cache, block_tables, context_lens)
    runner = _run or (lambda nc_, maps: run_bass_kernel_spmd(
        nc_, maps, core_ids=list(range(N_CORES))).results)
    results = runner(nc, in_maps)

    out = np.empty((B, H * D), np.float32)
    for c in range(N_CORES):
        row = results[c]["out"].reshape(SLOTS * H * D)
        for s in range(SLOTS):
            i = seq_ids_by_core[c][s]
            out[i] = row[s * H * D:(s + 1) * H * D]
    return out
